# revision 16
# baseline (speedup 1.0000x reference)
"""Trainium2 Bass kernel for nn_DenseModel_51926154609008 (weighted-rank
contrastive CE loss) — fp8 DoubleRow, chunked-DMA edition.

Math (reference semantics, no sort needed):
  scores = q @ p.T                       [B=2048, P=16384]
  t_i    = scores[i, 8*i]                (positive/target score, exact fp32
                                          on host)
  rank_i = #{j : scores[i, j] > t_i}     (argsort position == exceed count)
  lse_i  = logsumexp(scores[i, :])
  loss   = mean((lse_i - t_i) * (1 + 2.6*exp(-(rank_i-1)^2 / (2*1.8^2))))

Sharding: passage-parallel (P split across 8 cores, q replicated).

fp8 strategy: q, p quantized host-side to e4m3.  PE runs DoubleRow
(2 fp8 k-chunks per instruction, 216 ns per [128q x 512p x 256k] matmul
= the fp8 streaming peak).  192 matmuls/core = 41.5 us of PE floor.

v2 layout (startup-latency optimized):
 - p is chunked in DRAM as [4 col-chunks][3 k-pairs][128, 2, 512]
   (128 KB per chunk-pair, one plain 2-D DMA each), q as per-2-m-tile
   column chunks [128, 3, 2, 256] (192 KB), so the first matmul's
   operands are ~100 KB, spread over 5 DMA queues, landing ~10.5 us
   instead of 15.2 us (full-plane granularity).
 - ~20 dummy 128-wide matmuls on zeroed SBUF warm the PE p-state clock
   during the DMA window (the ramp otherwise costs ~2.5 us of 2x-slow
   real matmuls).
 - phase 1 runs ALL of m-tile 0 (chunk-gated, c-inner per bank), then
   all of m-tile 1, so m0's ACT (PSUM release) completes before m2
   needs the banks -- removes the 1.75 us double-buffer transition
   stall.
 - stats tile reordered so m15's 4 columns sit at the end: one early
   [128,30] output DMA (overlapped) + one tiny [128,4] final DMA.

m-major consumer structure (one [128, 2048] 4-bank PSUM tile per query
m-tile, double buffered over the 8 banks): 12 DoubleRow matmuls
(2.59 us) fill a tile; one 2048-wide ACTIVATE Exp (2.25 us incl. fused
accumulator read) produces the slab sumexp AND a bf16 exp tile je; one
2048-wide DVE count (2.43 us) compares je > theta_i = exp(t_i - C)
(exp is monotone; bf16 rounding only flips |s - t| <~ 2^-9 which is
noise vs the fp8 error).  All under the PE's 2.59 us -> PE-bound.

Self-column masking: inputs are rotated per-core so own queries land at
m-tiles 8, 9; those two counts use scalar_tensor_tensor with a bf16 0/1
mask generated on-device by a GpSimd iota.

Host combines per-m-tile partials and evaluates the tiny [2048] tail in
fp64.
"""

import sys

import numpy as np

sys.path.insert(0, "/opt/trn_rl_repo")

import concourse.bacc as bacc  # noqa: E402
import concourse.bass as bass  # noqa: E402
import concourse.mybir as mybir  # noqa: E402
import concourse.tile as tile  # noqa: E402
from concourse.bass_utils import run_bass_kernel_spmd  # noqa: E402

# Problem shape (hardcoded per the task contract).
B = 2048
D = 768
NP = 8
P = B * NP  # 16384
NCORES = 8
PSLAB = P // NCORES  # 2048 passage columns per core
KCH = D // 128  # 6 contraction chunks
KP = KCH // 2  # 3 DoubleRow chunk-pairs
MT = B // 128  # 16 query m-tiles
QSLAB = B // NCORES  # 256 queries owned per core
OWN_M = 8  # own queries sit at m-tiles 8,9
NWARM = 20  # PE p-state warmup matmuls

C_SHIFT = 128.0  # fixed exp shift: exp(s - C) never overflows

ALPHA = 2.6
OPTIMAL_RANK = 1.0
SIGMA = 1.8

_STATE: dict = {}


def _build_nc():
    nc = bacc.Bacc("TRN2", target_bir_lowering=False, debug=False,
                   num_devices=NCORES)

    f32 = mybir.dt.float32
    bf16 = mybir.dt.bfloat16
    fp8 = mybir.dt.float8e4

    # DRAM layouts (chunk-major, every DMA a plain 2-D [128, bytes] plane):
    #  qA: [2, 128, KP, 2, 128]   m-tile 0 / 1 column chunks (96 KB each)
    #  qB: [7, 128, KP, 2, 256]   m-tiles 2..15, one chunk per 2 m-tiles
    #  pC: [4, KP, 128, 2, 512]   col-chunk j, k-pair c -> 128 KB plane
    qA_d = nc.dram_tensor("qA", [2, 128, KP, 2, 128], fp8,
                          kind="ExternalInput").ap()
    qB_d = nc.dram_tensor("qB", [7, 128, KP, 2, 256], fp8,
                          kind="ExternalInput").ap()
    pC_d = nc.dram_tensor("pC", [4, KP, 128, 2, 512], fp8,
                          kind="ExternalInput").ap()
    th_d = nc.dram_tensor("thv", [128, MT + 1], f32,
                          kind="ExternalInput").ap()
    # stats output layout (per 128-row, col-major by m-tile):
    #   cols 0:15  = sumexp m0..m14        cols 15:30 = count m0..m14
    #   col 30,31  = sumexp m15 lo/hi half col 32,33  = count m15 lo/hi
    st_d = nc.dram_tensor("st_out", [128, 34], f32,
                          kind="ExternalOutput").ap()

    with tile.TileContext(nc) as tc:
        with (
            tc.tile_pool(name="weights", bufs=1) as wpool,
            tc.tile_pool(name="stats", bufs=1) as spool,
            tc.tile_pool(name="je", bufs=4) as jepool,
            tc.tile_pool(name="psum", bufs=2,
                         space=bass.MemorySpace.PSUM) as ppool,
        ):
            # --- SBUF tiles -------------------------------------------------
            qa = [wpool.tile([128, KP, 2, 128], fp8, name=f"qa{m}",
                             tag=f"qa{m}") for m in range(2)]
            qb = [wpool.tile([128, KP, 2, 256], fp8, name=f"qb{j}",
                             tag=f"qb{j}") for j in range(7)]
            # p chunk-pair tiles: pch[j][c] covers cols 512j..512j+512 of
            # the slab for k-pair c
            pch = [[wpool.tile([128, 2, 512], fp8, name=f"p{j}{c}",
                               tag=f"p{j}{c}") for c in range(KP)]
                   for j in range(4)]
            # warmup operands (zeroed; results are discarded)
            wq = wpool.tile([128, 2, 128], fp8, name="wq", tag="wq")
            wp = wpool.tile([128, 2, 512], fp8, name="wp", tag="wp")
            # cols 0:16 = theta per m-tile; col 16 = raw t (m15 psum count)
            thv = spool.tile([128, MT + 1], f32, name="thv", tag="thv")
            # one wide mask W[r, c] = (c - 8r - 1024 != 0), so
            # msk9 = W[:, 0:2048] (zero at 1024+8r) and
            # msk8 = W[:, 1024:3072] (zero at 8r within the slice)
            mskw = spool.tile([128, PSLAB + 1024], bf16, name="mskw",
                              tag="mskw")
            it16 = spool.tile([128, PSLAB + 1024], mybir.dt.int16,
                              name="it16", tag="it16")
            # per-engine stats tiles: the accumulator-flush writes would
            # otherwise form a cross-engine WAW chain through one tile,
            # serializing every ACT read behind the previous count read
            st_se = spool.tile([128, 18], f32, name="st_se", tag="st_se")
            st_ct = spool.tile([128, 18], f32, name="st_ct", tag="st_ct")
            negc = spool.tile([128, 1], f32, name="negc", tag="negc")

            # --- warmup + DMA schedule -------------------------------------
            # Only sync/scalar/gpsimd can issue DMAs.  Each p column-chunk
            # j has its 3 k-pairs spread across the 3 queues at queue
            # position ~j, so bank j's operands land in consumption order.
            nc.vector.memset(negc[:], -C_SHIFT)
            nc.vector.memset(wq[:], 0)
            nc.vector.memset(wp[:], 0)
            # Sync: pair-0 chunks + p22, then q bulk
            nc.sync.dma_start(pch[0][0][:], pC_d[0, 0])
            nc.sync.dma_start(pch[1][0][:], pC_d[1, 0])
            nc.sync.dma_start(pch[2][0][:], pC_d[2, 0])
            nc.sync.dma_start(pch[3][0][:], pC_d[3, 0])
            nc.sync.dma_start(pch[2][2][:], pC_d[2, 2])
            nc.sync.dma_start(qb[0][:], qB_d[0])
            nc.sync.dma_start(qb[2][:], qB_d[2])
            nc.sync.dma_start(qb[4][:], qB_d[4])
            nc.sync.dma_start(qb[6][:], qB_d[6])
            # Scalar: pair-1 chunks + p32 + qa1 + thv + q bulk
            nc.scalar.dma_start(pch[0][1][:], pC_d[0, 1])
            nc.scalar.dma_start(pch[1][1][:], pC_d[1, 1])
            nc.scalar.dma_start(pch[2][1][:], pC_d[2, 1])
            nc.scalar.dma_start(pch[3][1][:], pC_d[3, 1])
            nc.scalar.dma_start(pch[3][2][:], pC_d[3, 2])
            nc.scalar.dma_start(qa[1][:], qA_d[1])
            nc.scalar.dma_start(thv[:], th_d[:])
            nc.scalar.dma_start(qb[1][:], qB_d[1])
            nc.scalar.dma_start(qb[3][:], qB_d[3])
            nc.scalar.dma_start(qb[5][:], qB_d[5])
            # GpSimd (slow ring): qa0 + the two earliest pair-2 chunks + iota
            nc.gpsimd.dma_start(qa[0][:], qA_d[0])
            nc.gpsimd.dma_start(pch[0][2][:], pC_d[0, 2])
            nc.gpsimd.dma_start(pch[1][2][:], pC_d[1, 2])
            nc.gpsimd.iota(it16[:], [[1, PSLAB + 1024]], base=-1024,
                           channel_multiplier=-8)

            dr = mybir.MatmulPerfMode.DoubleRow

            ps01 = [ppool.tile([128, PSLAB], f32, name="ps", tag="ps")
                    for _ in range(2)]
            # PE p-state warmups: self-contained matmuls on zeros into psum
            # that m0's first real matmul (start=True) will reset.  Wide
            # (512) to accumulate continuous busy-time so the DVFS clock is
            # at full speed when the first operands land (~12.5 us); a few
            # small ones at the end for fine granularity.
            for _ in range(6):
                nc.tensor.matmul(ps01[0][:, 0:512], wq[:], wp[:],
                                 start=True, stop=True, perf_mode=dr,
                                 skip_group_check=True)
            for _ in range(2):
                nc.tensor.matmul(ps01[0][:, 0:128], wq[:], wp[:, :, 0:128],
                                 start=True, stop=True, perf_mode=dr,
                                 skip_group_check=True)

            # mask from iota (Vector; before the counts start)
            nc.vector.tensor_scalar(mskw[:], it16[:], 0, None,
                                    op0=mybir.AluOpType.not_equal)

            def qw(m, c):
                """Stationary (weights) AP for m-tile m, k-pair c."""
                if m < 2:
                    return qa[m][:, c, :, :]
                j = m // 2 - 1
                h = (m % 2) * 128
                return qb[j][:, c, :, h:h + 128]

            def mm(ps, m, b, c):
                nc.tensor.matmul(
                    ps[:, b * 512:(b + 1) * 512],
                    qw(m, c),
                    pch[b][c][:],
                    start=(c == 0),
                    stop=(c == KP - 1),
                    perf_mode=dr,
                )

            def consume(m, lo, hi, se_col, cnt_col):
                """Exp+sumexp (Scalar) and rank count (Vector) for
                ps[:, lo:hi] of m-tile m."""
                sl = slice(lo, hi)
                nc.scalar.activation(
                    je[:, sl], ps[:, sl], mybir.ActivationFunctionType.Exp,
                    bias=negc[:], scale=1.0,
                    accum_out=st_se[:, se_col:se_col + 1],
                )
                # the count overwrites je in place: je is dead after it
                # (sumexp comes from the ACT accumulator)
                if m in (OWN_M, OWN_M + 1):
                    off = 1024 if m == OWN_M else 0
                    nc.vector.scalar_tensor_tensor(
                        out=je[:, sl], in0=je[:, sl],
                        scalar=thv[:, m:m + 1],
                        in1=mskw[:, off + lo:off + hi],
                        op0=mybir.AluOpType.is_gt,
                        op1=mybir.AluOpType.mult,
                        accum_out=st_ct[:, cnt_col:cnt_col + 1],
                    )
                else:
                    nc.vector.tensor_scalar(
                        je[:, sl], je[:, sl], thv[:, m:m + 1], None,
                        op0=mybir.AluOpType.is_gt,
                        op1=mybir.AluOpType.add,
                        accum_out=st_ct[:, cnt_col:cnt_col + 1],
                    )

            # --- phase 1: m0 fully (chunk-gated), then m1 ------------------
            for m in range(2):
                ps = ps01[m]
                for b in range(4):
                    for c in range(KP):
                        mm(ps, m, b, c)
                je = jepool.tile([128, PSLAB], bf16, name="je", tag="je")
                consume(m, 0, 2048, m, m)

            # --- steady m-major loop ---------------------------------------
            for m in range(2, MT):
                ps = ppool.tile([128, PSLAB], f32, name="ps", tag="ps")
                for b in range(4):
                    for c in range(KP):
                        mm(ps, m, b, c)
                # m15 gets its own je slot so its ACT has no WAR on a
                # wrapped je buffer (that wait cost ~1.2us on the tail)
                je = jepool.tile([128, PSLAB], bf16, name="je",
                                 tag="je15" if m == MT - 1 else "je",
                                 bufs=1 if m == MT - 1 else None)
                if m == MT - 1:
                    # tail: count the hi half straight from PSUM fp32
                    # against the raw target (exact compare, and it starts
                    # the moment the matmuls finish instead of waiting for
                    # its ACT), junk output into the dead it16 tile; the lo
                    # half goes through the usual je path.
                    nc.vector.tensor_scalar(
                        it16[:, 0:1024], ps[:, 1024:2048],
                        thv[:, MT:MT + 1], None,
                        op0=mybir.AluOpType.is_gt,
                        op1=mybir.AluOpType.add,
                        accum_out=st_ct[:, 16:17],
                    )
                    consume(m, 0, 1024, 15, 15)
                    nc.scalar.activation(
                        je[:, 1024:2048], ps[:, 1024:2048],
                        mybir.ActivationFunctionType.Exp,
                        bias=negc[:], scale=1.0,
                        accum_out=st_se[:, 16:17],
                    )
                else:
                    consume(m, 0, 2048, m, m)

            # early bulk stats out (waits m0..m14 consumers), tiny finals
            nc.gpsimd.dma_start(st_d[:, 0:15], st_se[:, 0:15])
            nc.gpsimd.dma_start(st_d[:, 15:30], st_ct[:, 0:15])
            nc.gpsimd.dma_start(st_d[:, 30:32], st_se[:, 15:17])
            nc.gpsimd.dma_start(st_d[:, 32:34], st_ct[:, 15:17])

    nc.compile()
    return nc


def _perm(c):
    """Rotation putting core c's own queries at m-tiles OWN_M, OWN_M+1."""
    return np.roll(np.arange(B), OWN_M * 128 - c * QSLAB)


def prepare(q, p):
    """Host-side shard prep. Returns (in_maps, t32, perms)."""
    import ml_dtypes
    fp8 = ml_dtypes.float8_e4m3
    q = np.ascontiguousarray(np.asarray(q, dtype=np.float32))
    p = np.ascontiguousarray(np.asarray(p, dtype=np.float32))

    # target scores t_i = q_i . p_{8i} (exact fp32; threshold + host tail)
    t32 = np.einsum("ij,ij->i", q, p[::NP], dtype=np.float64).astype(np.float32)
    # count threshold in exp space: theta_i = exp(t_i - C); underflow to 0
    # only affects queries whose rank is huge (weight exactly 1) either way
    th32 = np.exp(t32.astype(np.float64) - C_SHIFT).astype(np.float32)

    q8 = q.astype(fp8)  # [B, D]
    p8 = p.astype(fp8)  # [P, D]
    # [KP, 2, 128, B]: pair c plane kk holds k-rows (2c+kk)*128..+127
    qT8 = np.ascontiguousarray(q8.T.reshape(KP, 2, 128, B))

    in_maps = []
    perms = []
    for c in range(NCORES):
        perm = _perm(c)
        perms.append(perm)
        qTc = qT8[:, :, :, perm]                       # [KP, 2, 128, B]
        # chunk-major: [128, KP, 2, B] -> column chunks
        qPM = np.ascontiguousarray(qTc.transpose(2, 0, 1, 3))
        qAc = np.stack([qPM[:, :, :, m * 128:(m + 1) * 128]
                        for m in range(2)])            # [2,128,KP,2,128]
        qBc = np.stack([qPM[:, :, :, 256 + j * 256: 512 + j * 256]
                        for j in range(7)])            # [7,128,KP,2,256]
        pT = p8[c * PSLAB:(c + 1) * PSLAB].T           # [D, PSLAB]
        pT8 = pT.reshape(KP, 2, 128, PSLAB)
        pCc = np.stack([
            np.stack([pT8[kp, :, :, j * 512:(j + 1) * 512]
                      .transpose(1, 0, 2) for kp in range(KP)])
            for j in range(4)])                        # [4,KP,128,2,512]
        thc = np.empty((128, MT + 1), np.float32)
        thc[:, :MT] = th32[perm].reshape(MT, 128).T
        thc[:, MT] = t32[perm].reshape(MT, 128).T[:, MT - 1]  # raw t, m15
        in_maps.append({
            "qA": np.ascontiguousarray(qAc),
            "qB": np.ascontiguousarray(qBc),
            "pC": np.ascontiguousarray(pCc),
            "thv": thc,
        })
    return in_maps, t32, perms


def finalize(results, t32, perms):
    """Combine per-core partials into the scalar loss (fp64 host tail)."""
    se_tot = np.zeros(B, dtype=np.float64)
    cnt_tot = np.zeros(B, dtype=np.float64)
    for c in range(NCORES):
        perm = perms[c]
        st = results[c]["st_out"].astype(np.float64)
        se = np.concatenate(
            [st[:, 0:15], (st[:, 30] + st[:, 31])[:, None]], axis=1)
        cnt = np.concatenate(
            [st[:, 15:30], (st[:, 32] + st[:, 33])[:, None]], axis=1)
        se_tot[perm] += se.T.ravel()
        cnt_tot[perm] += cnt.T.ravel()
    lse = C_SHIFT + np.log(se_tot)
    raw = lse - t32.astype(np.float64)
    w = 1.0 + ALPHA * np.exp(-((cnt_tot - OPTIMAL_RANK) ** 2)
                             / (2.0 * SIGMA ** 2))
    return np.float32(np.mean(raw * w))


def _get_nc():
    if "nc" not in _STATE:
        _STATE["nc"] = _build_nc()
    return _STATE["nc"]


def kernel(q_reps, p_reps, n_passages):
    assert int(np.asarray(n_passages)) == NP
    nc = _get_nc()
    in_maps, t32, perms = prepare(q_reps, p_reps)
    # rare transient NRT_EXEC_UNIT_UNRECOVERABLE: reset the PJRT client
    # and retry with backoff
    import time
    last = None
    for attempt in range(4):
        try:
            res = run_bass_kernel_spmd(nc, in_maps,
                                       core_ids=list(range(NCORES)))
            return finalize(res.results, t32, perms)
        except Exception as e:
            last = e
            try:
                import jax
                jax.clear_caches()
                jax.extend.backend.clear_backends()
            except Exception:
                pass
            time.sleep(10 * (attempt + 1))
    raise last


def run_profiled(q_reps, p_reps, n_passages, trace=True):
    """Same as kernel() but returns (loss, BassKernelResults) with NTFF
    profile (requires the antenv.axon_hooks shim; see _install_ntff_shim)."""
    nc = _get_nc()
    in_maps, t32, perms = prepare(q_reps, p_reps)
    res = run_bass_kernel_spmd(nc, in_maps, core_ids=list(range(NCORES)),
                               trace=trace)
    loss = finalize(res.results, t32, perms)
    return loss, res


def _install_ntff_shim():
    """Provide antenv.axon_hooks (absent in this image) so trace=True works."""
    import types
    import antenv
    if "antenv.axon_hooks" in sys.modules:
        return
    mod = types.ModuleType("antenv.axon_hooks")
    mod._hook = None
    mod.set_axon_ntff_profile_hook = lambda h: setattr(mod, "_hook", h)
    mod.get_axon_ntff_profile_hook = lambda: mod._hook
    sys.modules["antenv.axon_hooks"] = mod
    antenv.axon_hooks = mod
    try:
        from trn_agent_boot.trn_boot import _ntff_profile_via_ctypes
        hook = _ntff_profile_via_ctypes("/opt/axon/libaxon_pjrt.so")
        if hook is not None:
            mod._hook = hook
    except Exception:
        pass
